# revision 1
# baseline (speedup 1.0000x reference)
"""Fused RBF-kernel-PCA loss on 8 Trainium2 NeuronCores.

Math: K[i,j] = exp((x_i.x_j - |x_i|^2/2 - |x_j|^2/2) / sigma2), E = W^T K,
loss1 = -1/2 sum_s lam_s sum_j E[s,j]^2, loss2 = 1/2 sum_{s,j} E[s,j] W[j,s],
L = loss1 + loss2, out = L + 0.05 L^2.

Sharding: core c owns j-columns [c*1024, (c+1)*1024) of E and computes them
fully (contraction over all i happens on-core via PSUM accumulation), so no
collectives are needed; each core emits [64, 4] partial sums
(sum_j E^2 and sum_j E*W per s, per 512-wide panel) and the host folds the
8x[64,4] partials into the scalar loss.
"""

import numpy as np

import concourse.bass as bass
import concourse.bacc as bacc
import concourse.mybir as mybir
from concourse import tile
from concourse.bass_utils import run_bass_kernel_spmd

N, D, S = 8192, 512, 64
NCORES = 8
JPC = N // NCORES          # 1024 j-columns per core
PANW = 512                 # panel width (PSUM free-dim limit)
NPAN = JPC // PANW         # 2 panels per core
NIB = N // 128             # 64 i-blocks
NDC = D // 128             # 4 contraction chunks
SIGMA2 = 512.0
ETA = 1.0
C_STAB = 0.1

_nc_cache = {}


def build_nc(nib=NIB, npan=NPAN):
    key = (nib, npan)
    if key in _nc_cache:
        return _nc_cache[key]
    dt = mybir.dt.float32
    nc = bacc.Bacc(
        "TRN2", target_bir_lowering=False, debug=False, num_devices=NCORES
    )

    xT = nc.dram_tensor("xT", [NDC, 128, N], dt, kind="ExternalInput")
    xjT = nc.dram_tensor("xjT", [NDC, 128, JPC], dt, kind="ExternalInput")
    wT = nc.dram_tensor("wT", [128, NIB * S], dt, kind="ExternalInput")
    wcT = nc.dram_tensor("wcT", [S, JPC], dt, kind="ExternalInput")
    sqj = nc.dram_tensor("sqj", [128, JPC], dt, kind="ExternalInput")
    sqbias = nc.dram_tensor("sqbias", [128, NIB], dt, kind="ExternalInput")
    out = nc.dram_tensor("out", [S, 2 * NPAN], dt, kind="ExternalOutput")

    with tile.TileContext(nc) as tc:
        with (
            tc.tile_pool(name="const", bufs=1) as cpool,
            tc.tile_pool(name="ktile", bufs=3) as kpool,
            tc.tile_pool(name="utile", bufs=3) as upool,
            tc.tile_pool(name="esb", bufs=2) as epool,
            tc.tile_pool(name="scr", bufs=2) as spool,
            tc.tile_pool(name="pdot", bufs=3, space="PSUM") as pdot,
            tc.tile_pool(name="pe", bufs=2, space="PSUM") as pe_pool,
        ):
            xjt_sb = []
            for d in range(NDC):
                t = cpool.tile([128, JPC], dt, tag=f"xjt{d}")
                nc.sync.dma_start(t[:], xjT[d])
                xjt_sb.append(t)
            sqj_sb = cpool.tile([128, JPC], dt, tag="sqj")
            nc.sync.dma_start(sqj_sb[:], sqj[:])
            sqb_sb = cpool.tile([128, NIB], dt, tag="sqb")
            nc.sync.dma_start(sqb_sb[:], sqbias[:])
            wt_sb = cpool.tile([128, NIB * S], dt, tag="wt")
            nc.sync.dma_start(wt_sb[:], wT[:])
            wct_sb = cpool.tile([S, JPC], dt, tag="wct")
            nc.sync.dma_start(wct_sb[:], wcT[:])

            # x^T chunks: [128, 1024] per (d-chunk, column-chunk), loaded
            # column-chunk-major so early i-blocks are ready first.
            xt_sb = [[None] * NCORES for _ in range(NDC)]
            for cc in range(NCORES):
                for d in range(NDC):
                    t = cpool.tile([128, JPC], dt, tag=f"xt{d}_{cc}")
                    nc.sync.dma_start(t[:], xT[d, :, cc * JPC:(cc + 1) * JPC])
                    xt_sb[d][cc] = t

            out_sb = cpool.tile([S, 2 * NPAN], dt, tag="out")

            for panel in range(npan):
                psum_e = pe_pool.tile([S, PANW], dt)
                emm = []  # deferred E-accumulation matmuls (1-tile stagger)
                for ib in range(nib):
                    cc, ibc = divmod(ib, NCORES)
                    psum_d = pdot.tile([128, PANW], dt)
                    for d in range(NDC):
                        nc.tensor.matmul(
                            psum_d[:],
                            xt_sb[d][cc][:, ibc * 128:(ibc + 1) * 128],
                            xjt_sb[d][:, panel * PANW:(panel + 1) * PANW],
                            start=(d == 0),
                            stop=(d == NDC - 1),
                        )
                    # u = dot - sq_j/2  (sqj holds the negated term)
                    ut = upool.tile([128, PANW], dt)
                    nc.vector.tensor_add(
                        ut[:],
                        psum_d[:],
                        sqj_sb[:, panel * PANW:(panel + 1) * PANW],
                    )
                    # K = exp(u/sigma2 - sq_i/(2*sigma2))
                    kt = kpool.tile([128, PANW], dt)
                    nc.scalar.activation(
                        kt[:],
                        ut[:],
                        mybir.ActivationFunctionType.Exp,
                        bias=sqb_sb[:, ib:ib + 1],
                        scale=1.0 / SIGMA2,
                    )
                    emm.append((kt, ib))
                    if len(emm) >= 2:
                        ekt, eib = emm.pop(0)
                        nc.tensor.matmul(
                            psum_e[:],
                            wt_sb[:, eib * S:(eib + 1) * S],
                            ekt[:],
                            start=(eib == 0),
                            stop=(eib == nib - 1),
                            skip_group_check=True,
                        )
                for ekt, eib in emm:
                    nc.tensor.matmul(
                        psum_e[:],
                        wt_sb[:, eib * S:(eib + 1) * S],
                        ekt[:],
                        start=(eib == 0),
                        stop=(eib == nib - 1),
                        skip_group_check=True,
                    )

                # loss partials for this panel
                e_sb = epool.tile([S, PANW], dt)
                nc.scalar.activation(
                    e_sb[:], psum_e[:], mybir.ActivationFunctionType.Copy
                )
                scr1 = spool.tile([S, PANW], dt, tag="scr1")
                nc.vector.tensor_mul(scr1[:], e_sb[:], e_sb[:])
                nc.vector.reduce_sum(
                    out_sb[:, panel:panel + 1], scr1[:],
                    axis=mybir.AxisListType.X,
                )
                scr2 = spool.tile([S, PANW], dt, tag="scr2")
                nc.vector.tensor_mul(
                    scr2[:], e_sb[:],
                    wct_sb[:, panel * PANW:(panel + 1) * PANW],
                )
                nc.vector.reduce_sum(
                    out_sb[:, NPAN + panel:NPAN + panel + 1], scr2[:],
                    axis=mybir.AxisListType.X,
                )

            nc.sync.dma_start(out[:], out_sb[:])

    nc.finalize()
    _nc_cache[key] = nc
    return nc


def _prep_inputs(input_data, weight):
    x = np.ascontiguousarray(input_data, dtype=np.float32)
    w = np.ascontiguousarray(weight, dtype=np.float32)
    sq = np.einsum("nd,nd->n", x, x).astype(np.float32)
    nsq = (-sq / (2.0 * SIGMA2)).astype(np.float32)  # ACT bias term
    nsqh = (-sq / 2.0).astype(np.float32)            # pre-exp additive term

    xT = np.ascontiguousarray(x.T).reshape(NDC, 128, N)
    wT = np.ascontiguousarray(
        w.reshape(NIB, 128, S).transpose(1, 0, 2).reshape(128, NIB * S)
    )
    sqbias = np.ascontiguousarray(nsq.reshape(NIB, 128).T)

    in_maps = []
    for c in range(NCORES):
        jlo, jhi = c * JPC, (c + 1) * JPC
        in_maps.append({
            "xT": xT,
            "xjT": np.ascontiguousarray(xT[:, :, jlo:jhi]),
            "wT": wT,
            "wcT": np.ascontiguousarray(w[jlo:jhi].T),
            "sqj": np.ascontiguousarray(
                np.broadcast_to(nsqh[jlo:jhi], (128, JPC))
            ),
            "sqbias": sqbias,
        })
    return in_maps


def _combine(outs, inv_lambda_diag):
    r1 = np.zeros(S, dtype=np.float64)
    r2 = np.zeros(S, dtype=np.float64)
    for o in outs:
        o = o.astype(np.float64)
        r1 += o[:, :NPAN].sum(axis=1)
        r2 += o[:, NPAN:].sum(axis=1)
    lam = np.asarray(inv_lambda_diag, dtype=np.float64)
    loss1 = -float(np.dot(lam, r1)) / (2.0 * ETA**2)
    loss2 = float(r2.sum()) / (2.0 * ETA)
    L = loss1 + loss2
    return np.asarray(L + (C_STAB / 2.0) * L * L, dtype=np.float32)


def run(input_data, weight, inv_lambda_diag, **run_kwargs):
    nc = build_nc()
    in_maps = _prep_inputs(input_data, weight)
    res = run_bass_kernel_spmd(nc, in_maps, list(range(NCORES)), **run_kwargs)
    outs = [res.results[c]["out"] for c in range(NCORES)]
    return _combine(outs, inv_lambda_diag), res


def kernel(input_data, weight, inv_lambda_diag):
    ans, _ = run(input_data, weight, inv_lambda_diag)
    return ans



# revision 7
# speedup vs baseline: 3.8081x; 3.8081x over previous
"""Fused RBF-kernel-PCA loss on 8 Trainium2 NeuronCores (bf16 tensor path).

Math: K[i,j] = exp((x_i.x_j - |x_i|^2/2)/sigma2) * c_j with
c_j = exp(-|x_j|^2/(2*sigma2)), E = W^T K, loss1 = -1/2 sum_s lam_s sum_j
E[s,j]^2, loss2 = 1/2 sum_{s,j} E[s,j] W[j,s], L = loss1 + loss2,
out = L + 0.05 L^2.

Sharding: core c owns j-columns [c*1024, (c+1)*1024) of E and computes them
fully (contraction over all i happens on-core via PSUM accumulation); no
collectives. Each core emits r1[s] = sum_j (c_j Etil)^2 and
r2[s] = sum_j Etil[s,j] w[j,s] c_j; the host folds 8x[64,2] partials into
the scalar loss.

Perf structure vs the fp32 baseline (665us):
 - all matmuls in bf16 (1 PE cycle/row vs fp32's 4)
 - 1024-wide compound matmuls: one LDWEIGHTS per 2 MULTIPLYs, PSUM tiles
   span 2 banks
 - the -|x_j|^2/2 term is factored out of the exp (c_j), so no [128,1024]
   vector add per i-block; ACT reads PSUM directly and writes bf16
 - c_j / c_j^2 are folded into host-precomputed [64,1024] tensors applied
   once on the tiny E tile
 - w is split into bf16 hi+lo packed as one 128-column lhsT (precision of
   ~fp16+ on the W^T K reduction at zero extra MULTIPLY cost)
"""

import numpy as np
import ml_dtypes

import concourse.bass as bass
import concourse.bacc as bacc
import concourse.mybir as mybir
from concourse import tile
from concourse.bass_utils import run_bass_kernel_spmd

N, D, S = 8192, 512, 64
NCORES = 8
JPC = N // NCORES          # 1024 j-columns per core
NIB = N // 128             # 64 i-blocks
NDC = D // 128             # 4 contraction chunks
NCC = N // JPC             # 8 column-chunks of x^T over i
PANW = 512                 # matmul out free-size limit (one PSUM bank)
NPAN = JPC // PANW         # 2 panels per core
IBC = JPC // 128           # 8 i-blocks per column-chunk
SIGMA2 = 512.0
ETA = 1.0
C_STAB = 0.1

BF16 = np.dtype(ml_dtypes.bfloat16)

_nc_cache = {}


def build_nc():
    key = 0
    if key in _nc_cache:
        return _nc_cache[key]
    f32 = mybir.dt.float32
    bf16 = mybir.dt.bfloat16
    nc = bacc.Bacc(
        "TRN2", target_bir_lowering=False, debug=False, num_devices=NCORES
    )

    xT = nc.dram_tensor("xT", [NDC, 128, N], bf16, kind="ExternalInput")
    xjT = nc.dram_tensor("xjT", [NDC, 128, JPC], bf16, kind="ExternalInput")
    wT2 = nc.dram_tensor("wT2", [128, NIB * 128], bf16, kind="ExternalInput")
    wc2T = nc.dram_tensor("wc2T", [S, JPC], f32, kind="ExternalInput")
    c2bc = nc.dram_tensor("c2bc", [S, JPC], f32, kind="ExternalInput")
    sqbias = nc.dram_tensor("sqbias", [128, NIB], f32, kind="ExternalInput")
    out = nc.dram_tensor("out", [S, 2 * NPAN], f32, kind="ExternalOutput")

    with tile.TileContext(nc) as tc:
        with (
            tc.tile_pool(name="const", bufs=1) as cpool,
            tc.tile_pool(name="ktile", bufs=6) as kpool,
            tc.tile_pool(name="scr", bufs=1) as spool,
            tc.tile_pool(name="pdot", bufs=4, space="PSUM") as pdot,
            tc.tile_pool(name="pe", bufs=1, space="PSUM") as pe_pool,
        ):
            xjt_sb = []
            for d in range(NDC):
                t = cpool.tile([128, JPC], bf16, tag=f"xjt{d}")
                nc.sync.dma_start(t[:], xjT[d])
                xjt_sb.append(t)
            sqb_sb = cpool.tile([128, NIB], f32, tag="sqb")
            nc.sync.dma_start(sqb_sb[:], sqbias[:])
            wt_sb = cpool.tile([128, NIB * 128], bf16, tag="wt")
            nc.sync.dma_start(wt_sb[:], wT2[:])
            wc2t_sb = cpool.tile([S, JPC], f32, tag="wc2t")
            nc.sync.dma_start(wc2t_sb[:], wc2T[:])
            c2bc_sb = cpool.tile([S, JPC], f32, tag="c2bc")
            nc.sync.dma_start(c2bc_sb[:], c2bc[:])

            # x^T chunks: [128, 1024] per (d-chunk, column-chunk), loaded
            # column-chunk-major so early i-blocks are ready first.
            xt_sb = [[None] * NCC for _ in range(NDC)]
            for cc in range(NCC):
                for d in range(NDC):
                    t = cpool.tile([128, JPC], bf16, tag=f"xt{d}_{cc}")
                    nc.sync.dma_start(t[:], xT[d, :, cc * JPC:(cc + 1) * JPC])
                    xt_sb[d][cc] = t

            out_sb = cpool.tile([S, 2 * NPAN], f32, tag="out")

            # Panel-serial (baseline-proven shape): one open E-accumulation
            # group at a time, interleaved only with psum_d groups.
            psum_e = []
            for p in range(NPAN):
                pe = pe_pool.tile([128, PANW], f32, tag=f"pe{p}",
                                  name=f"psum_e{p}")
                psum_e.append(pe)
                emm = []  # deferred E-accumulation matmuls (1-tile stagger)

                def issue_emm(ekt, eib):
                    nc.tensor.matmul(
                        pe[:],
                        wt_sb[:, eib * 128:(eib + 1) * 128],
                        ekt[:],
                        start=(eib == 0),
                        stop=(eib == NIB - 1),
                        skip_group_check=True,
                    )

                for ib in range(NIB):
                    cc, ibc = divmod(ib, IBC)
                    psum_d = pdot.tile([128, PANW], f32)
                    for d in range(NDC):
                        nc.tensor.matmul(
                            psum_d[:],
                            xt_sb[d][cc][:, ibc * 128:(ibc + 1) * 128],
                            xjt_sb[d][:, p * PANW:(p + 1) * PANW],
                            start=(d == 0),
                            stop=(d == NDC - 1),
                        )
                    # Ktil = exp(dot/sigma2 - sq_i/(2*sigma2)) as bf16
                    kt = kpool.tile([128, PANW], bf16)
                    nc.scalar.activation(
                        kt[:],
                        psum_d[:],
                        mybir.ActivationFunctionType.Exp,
                        bias=sqb_sb[:, ib:ib + 1],
                        scale=1.0 / SIGMA2,
                    )
                    emm.append((kt, ib))
                    if len(emm) >= 2:
                        issue_emm(*emm.pop(0))
                for ekt, eib in emm:
                    issue_emm(ekt, eib)

            # E = hi + lo halves of the packed W^T K accumulation
            # (DVE may read only one PSUM operand: stage lo through SBUF)
            for p in range(NPAN):
                elo_sb = spool.tile([S, PANW], f32, tag=f"elo{p}")
                nc.scalar.activation(
                    elo_sb[:], psum_e[p][S:2 * S, :],
                    mybir.ActivationFunctionType.Copy,
                )
                e_sb = spool.tile([S, PANW], f32, tag=f"e{p}")
                nc.vector.tensor_add(e_sb[:], psum_e[p][0:S, :], elo_sb[:])
                # r1[s] = sum_j E^2 c_j^2 ; r2[s] = sum_j E (w c)_j
                e2_sb = spool.tile([S, PANW], f32, tag=f"e2{p}")
                nc.vector.tensor_mul(e2_sb[:], e_sb[:], e_sb[:])
                scr1 = spool.tile([S, PANW], f32, tag=f"scr1{p}")
                nc.vector.tensor_mul(
                    scr1[:], e2_sb[:], c2bc_sb[:, p * PANW:(p + 1) * PANW]
                )
                nc.vector.reduce_sum(
                    out_sb[:, p:p + 1], scr1[:], axis=mybir.AxisListType.X
                )
                scr2 = spool.tile([S, PANW], f32, tag=f"scr2{p}")
                nc.vector.tensor_mul(
                    scr2[:], e_sb[:], wc2t_sb[:, p * PANW:(p + 1) * PANW]
                )
                nc.vector.reduce_sum(
                    out_sb[:, NPAN + p:NPAN + p + 1], scr2[:],
                    axis=mybir.AxisListType.X
                )

            nc.sync.dma_start(out[:], out_sb[:])

    nc.finalize()
    _nc_cache[key] = nc
    return nc


def _prep_inputs(input_data, weight):
    x = np.ascontiguousarray(input_data, dtype=np.float32)
    w = np.ascontiguousarray(weight, dtype=np.float32)

    x16 = x.astype(BF16)
    x16f = x16.astype(np.float64)
    sq = np.einsum("nd,nd->n", x16f, x16f)          # |x16_i|^2, exact
    nsq = (-sq / (2.0 * SIGMA2)).astype(np.float32)  # ACT bias term
    cj = np.exp(-sq / (2.0 * SIGMA2))                # c_j, float64

    xT = np.ascontiguousarray(x16.T).reshape(NDC, 128, N)
    sqbias = np.ascontiguousarray(nsq.reshape(NIB, 128).T)

    # w split into bf16 hi+lo, packed [128, NIB*128]:
    # columns [ib*128, ib*128+64) = w_hi for block ib, +64.. = w_lo
    w_hi = w.astype(BF16)
    w_lo = (w - w_hi.astype(np.float32)).astype(BF16)
    packed = np.concatenate(
        [w_hi.reshape(NIB, 128, S), w_lo.reshape(NIB, 128, S)], axis=2
    )
    wT2 = np.ascontiguousarray(
        packed.transpose(1, 0, 2).reshape(128, NIB * 128)
    )

    in_maps = []
    for c in range(NCORES):
        jlo, jhi = c * JPC, (c + 1) * JPC
        cjc = cj[jlo:jhi]
        wc2T = np.ascontiguousarray((w[jlo:jhi] * cjc[:, None]).T)
        c2bc = np.ascontiguousarray(
            np.broadcast_to((cjc * cjc).astype(np.float32), (S, JPC))
        )
        in_maps.append({
            "xT": xT,
            "xjT": np.ascontiguousarray(xT[:, :, jlo:jhi]),
            "wT2": wT2,
            "wc2T": wc2T.astype(np.float32),
            "c2bc": c2bc,
            "sqbias": sqbias,
        })
    return in_maps


def _combine(outs, inv_lambda_diag):
    r1 = np.zeros(S, dtype=np.float64)
    r2 = np.zeros(S, dtype=np.float64)
    for o in outs:
        o = o.astype(np.float64)
        r1 += o[:, :NPAN].sum(axis=1)
        r2 += o[:, NPAN:].sum(axis=1)
    lam = np.asarray(inv_lambda_diag, dtype=np.float64)
    loss1 = -float(np.dot(lam, r1)) / (2.0 * ETA**2)
    loss2 = float(r2.sum()) / (2.0 * ETA)
    L = loss1 + loss2
    return np.asarray(L + (C_STAB / 2.0) * L * L, dtype=np.float32)


def run(input_data, weight, inv_lambda_diag, **run_kwargs):
    nc = build_nc()
    in_maps = _prep_inputs(input_data, weight)
    res = run_bass_kernel_spmd(nc, in_maps, list(range(NCORES)), **run_kwargs)
    outs = [res.results[c]["out"] for c in range(NCORES)]
    return _combine(outs, inv_lambda_diag), res


def kernel(input_data, weight, inv_lambda_diag):
    ans, _ = run(input_data, weight, inv_lambda_diag)
    return ans


# revision 8
# speedup vs baseline: 5.8635x; 1.5397x over previous
"""Fused RBF-kernel-PCA loss on 8 Trainium2 NeuronCores (fp8 tensor path).

Math: K[i,j] = exp((x_i.x_j - |x_i|^2/2)/sigma2) * c_j with
c_j = exp(-|x_j|^2/(2*sigma2)), E = W^T K, loss1 = -1/2 sum_s lam_s sum_j
E[s,j]^2, loss2 = 1/2 sum_{s,j} E[s,j] W[j,s], L = loss1 + loss2,
out = L + 0.05 L^2.

Sharding: core c owns j-columns [c*1024, (c+1)*1024) of E and computes them
fully (contraction over all i happens on-core via PSUM accumulation); no
collectives. Each core emits r1[s] = sum_j (c_j Etil)^2 and
r2[s] = sum_j Etil[s,j] w[j,s] c_j; the host folds 8x[64,4] partials into
the scalar loss.

Perf structure (vs 665us fp32 baseline, 175us bf16 v1):
 - x.x^T in fp8 e4m3 with DoubleRow perf mode: 0.5 PE cycles/row and a
   256-deep contraction per instruction (4x fewer tensor cycles than bf16).
   sq/c_j/bias are computed on host from the fp8-dequantized x, so the
   device evaluates the EXACT rbf kernel of the perturbed points - the fp8
   error acts as a tiny input perturbation, not a dot-product error.
 - W^T K stays bf16 with w split hi+lo packed as one 128-column lhsT
   (~fp16 precision at no extra MULTIPLY cost; out rows 0-63 = E_hi
   contribution, 64-127 = E_lo).
 - one 1024-wide exp ACTIVATE per i-block straight out of PSUM (bias =
   -|x_i|^2/(2*sigma2) per partition), writing bf16.
 - the -|x_j|^2/2 term is factored out of the exp (c_j) and folded into
   host-precomputed [64,1024] tensors applied once on the tiny E tile.
"""

import numpy as np
import ml_dtypes

import concourse.bass as bass
import concourse.bacc as bacc
import concourse.mybir as mybir
from concourse import tile
from concourse.bass_utils import run_bass_kernel_spmd

N, D, S = 8192, 512, 64
NCORES = 8
JPC = N // NCORES          # 1024 j-columns per core
NIB = N // 128             # 64 i-blocks
NHALF = 2                  # 256-deep DoubleRow contraction chunks
NCC = N // JPC             # 8 column-chunks of x^T over i
PANW = 512                 # matmul out free-size limit (one PSUM bank)
NPAN = JPC // PANW         # 2 panels per core
IBC = JPC // 128           # 8 i-blocks per column-chunk
SIGMA2 = 512.0
ETA = 1.0
C_STAB = 0.1

BF16 = np.dtype(ml_dtypes.bfloat16)
FP8 = np.dtype(ml_dtypes.float8_e4m3)

_nc_cache = {}


def build_nc():
    key = 0
    if key in _nc_cache:
        return _nc_cache[key]
    f32 = mybir.dt.float32
    bf16 = mybir.dt.bfloat16
    fp8 = mybir.dt.float8e4
    DR = mybir.MatmulPerfMode.DoubleRow
    nc = bacc.Bacc(
        "TRN2", target_bir_lowering=False, debug=False, num_devices=NCORES
    )

    xT = nc.dram_tensor("xT", [NHALF, 128, 2, N], fp8, kind="ExternalInput")
    xjT = nc.dram_tensor("xjT", [NHALF, 128, 2, JPC], fp8,
                         kind="ExternalInput")
    wT2 = nc.dram_tensor("wT2", [128, NIB * 128], bf16, kind="ExternalInput")
    wc2T = nc.dram_tensor("wc2T", [S, JPC], f32, kind="ExternalInput")
    c2bc = nc.dram_tensor("c2bc", [S, JPC], f32, kind="ExternalInput")
    sqbias = nc.dram_tensor("sqbias", [128, NIB], f32, kind="ExternalInput")
    out = nc.dram_tensor("out", [S, 2 * NPAN], f32, kind="ExternalOutput")

    with tile.TileContext(nc) as tc:
        with (
            tc.tile_pool(name="const", bufs=1) as cpool,
            tc.tile_pool(name="ktile", bufs=4) as kpool,
            tc.tile_pool(name="scr", bufs=1) as spool,
            tc.tile_pool(name="pdot", bufs=3, space="PSUM") as pdot,
            tc.tile_pool(name="pe", bufs=1, space="PSUM") as pe_pool,
        ):
            xjt_sb = []
            for h in range(NHALF):
                t = cpool.tile([128, 2, JPC], fp8, tag=f"xjt{h}",
                               name=f"xjt{h}")
                nc.sync.dma_start(t[:], xjT[h])
                xjt_sb.append(t)
            sqb_sb = cpool.tile([128, NIB], f32, tag="sqb")
            nc.sync.dma_start(sqb_sb[:], sqbias[:])

            # x^T chunks: [128, 2, 1024] per (half, column-chunk), loaded
            # column-chunk-major so early i-blocks are ready first; cc0
            # before the big w tile so compute starts ASAP.
            xt_sb = [[None] * NCC for _ in range(NHALF)]

            def load_cc(cc):
                for h in range(NHALF):
                    t = cpool.tile([128, 2, JPC], fp8, tag=f"xt{h}_{cc}",
                                   name=f"xt{h}_{cc}")
                    nc.sync.dma_start(
                        t[:], xT[h][:, :, cc * JPC:(cc + 1) * JPC]
                    )
                    xt_sb[h][cc] = t

            load_cc(0)
            wt_sb = cpool.tile([128, NIB * 128], bf16, tag="wt")
            nc.sync.dma_start(wt_sb[:], wT2[:])
            load_cc(1)
            wc2t_sb = cpool.tile([S, JPC], f32, tag="wc2t")
            nc.sync.dma_start(wc2t_sb[:], wc2T[:])
            c2bc_sb = cpool.tile([S, JPC], f32, tag="c2bc")
            nc.sync.dma_start(c2bc_sb[:], c2bc[:])
            for cc in range(2, NCC):
                load_cc(cc)

            out_sb = cpool.tile([S, 2 * NPAN], f32, tag="out")

            psum_e = []
            for p in range(NPAN):
                pe = pe_pool.tile([128, PANW], f32, tag=f"pe{p}",
                                  name=f"psum_e{p}")
                psum_e.append(pe)

            def issue_emms(kts, eib):
                for p in range(NPAN):
                    nc.tensor.matmul(
                        psum_e[p][:],
                        wt_sb[:, eib * 128:(eib + 1) * 128],
                        kts[:, p * PANW:(p + 1) * PANW],
                        start=(eib == 0),
                        stop=(eib == NIB - 1),
                        skip_group_check=True,
                    )

            prev = None
            for ib in range(NIB):
                cc, ibc = divmod(ib, IBC)
                psum_d = pdot.tile([128, NPAN * PANW], f32)
                for p in range(NPAN):
                    for h in range(NHALF):
                        nc.tensor.matmul(
                            psum_d[:, p * PANW:(p + 1) * PANW],
                            xt_sb[h][cc][:, :, ibc * 128:(ibc + 1) * 128],
                            xjt_sb[h][:, :, p * PANW:(p + 1) * PANW],
                            start=(h == 0),
                            stop=(h == NHALF - 1),
                            perf_mode=DR,
                        )
                # Ktil = exp(dot/sigma2 - sq_i/(2*sigma2)) as bf16, 1024 wide
                kt = kpool.tile([128, NPAN * PANW], bf16)
                nc.scalar.activation(
                    kt[:],
                    psum_d[:],
                    mybir.ActivationFunctionType.Exp,
                    bias=sqb_sb[:, ib:ib + 1],
                    scale=1.0 / SIGMA2,
                )
                if prev is not None:
                    issue_emms(*prev)
                prev = (kt, ib)
            issue_emms(*prev)

            # E = hi + lo halves of the packed W^T K accumulation
            # (DVE may read only one PSUM operand: stage lo through SBUF)
            for p in range(NPAN):
                elo_sb = spool.tile([S, PANW], f32, tag=f"elo{p}")
                nc.scalar.activation(
                    elo_sb[:], psum_e[p][S:2 * S, :],
                    mybir.ActivationFunctionType.Copy,
                )
                e_sb = spool.tile([S, PANW], f32, tag=f"e{p}")
                nc.vector.tensor_add(e_sb[:], psum_e[p][0:S, :], elo_sb[:])
                # r1[s] = sum_j E^2 c_j^2 ; r2[s] = sum_j E (w c)_j
                e2_sb = spool.tile([S, PANW], f32, tag=f"e2{p}")
                nc.vector.tensor_mul(e2_sb[:], e_sb[:], e_sb[:])
                scr1 = spool.tile([S, PANW], f32, tag=f"scr1{p}")
                nc.vector.tensor_mul(
                    scr1[:], e2_sb[:], c2bc_sb[:, p * PANW:(p + 1) * PANW]
                )
                nc.vector.reduce_sum(
                    out_sb[:, p:p + 1], scr1[:], axis=mybir.AxisListType.X
                )
                scr2 = spool.tile([S, PANW], f32, tag=f"scr2{p}")
                nc.vector.tensor_mul(
                    scr2[:], e_sb[:], wc2t_sb[:, p * PANW:(p + 1) * PANW]
                )
                nc.vector.reduce_sum(
                    out_sb[:, NPAN + p:NPAN + p + 1], scr2[:],
                    axis=mybir.AxisListType.X
                )

            nc.sync.dma_start(out[:], out_sb[:])

    nc.finalize()
    _nc_cache[key] = nc
    return nc


def _prep_inputs(input_data, weight):
    x = np.ascontiguousarray(input_data, dtype=np.float32)
    w = np.ascontiguousarray(weight, dtype=np.float32)

    x8 = x.astype(FP8)
    x8f = x8.astype(np.float64)
    sq = np.einsum("nd,nd->n", x8f, x8f)             # |x8_i|^2, exact
    nsq = (-sq / (2.0 * SIGMA2)).astype(np.float32)  # ACT bias term
    cj = np.exp(-sq / (2.0 * SIGMA2))                # c_j, float64

    # x^T as [half, p, t, n] with d = half*256 + t*128 + p
    xT = np.ascontiguousarray(
        x8.T.reshape(NHALF, 2, 128, N).transpose(0, 2, 1, 3)
    )
    sqbias = np.ascontiguousarray(nsq.reshape(NIB, 128).T)

    # w split into bf16 hi+lo, packed [128, NIB*128]:
    # columns [ib*128, ib*128+64) = w_hi for block ib, +64.. = w_lo
    w_hi = w.astype(BF16)
    w_lo = (w - w_hi.astype(np.float32)).astype(BF16)
    packed = np.concatenate(
        [w_hi.reshape(NIB, 128, S), w_lo.reshape(NIB, 128, S)], axis=2
    )
    wT2 = np.ascontiguousarray(
        packed.transpose(1, 0, 2).reshape(128, NIB * 128)
    )

    in_maps = []
    for c in range(NCORES):
        jlo, jhi = c * JPC, (c + 1) * JPC
        cjc = cj[jlo:jhi]
        wc2T = np.ascontiguousarray((w[jlo:jhi] * cjc[:, None]).T)
        c2bc = np.ascontiguousarray(
            np.broadcast_to((cjc * cjc).astype(np.float32), (S, JPC))
        )
        in_maps.append({
            "xT": xT,
            "xjT": np.ascontiguousarray(xT[:, :, :, jlo:jhi]),
            "wT2": wT2,
            "wc2T": wc2T.astype(np.float32),
            "c2bc": c2bc,
            "sqbias": sqbias,
        })
    return in_maps


def _combine(outs, inv_lambda_diag):
    r1 = np.zeros(S, dtype=np.float64)
    r2 = np.zeros(S, dtype=np.float64)
    for o in outs:
        o = o.astype(np.float64)
        r1 += o[:, :NPAN].sum(axis=1)
        r2 += o[:, NPAN:].sum(axis=1)
    lam = np.asarray(inv_lambda_diag, dtype=np.float64)
    loss1 = -float(np.dot(lam, r1)) / (2.0 * ETA**2)
    loss2 = float(r2.sum()) / (2.0 * ETA)
    L = loss1 + loss2
    return np.asarray(L + (C_STAB / 2.0) * L * L, dtype=np.float32)


def run(input_data, weight, inv_lambda_diag, **run_kwargs):
    nc = build_nc()
    in_maps = _prep_inputs(input_data, weight)
    res = run_bass_kernel_spmd(nc, in_maps, list(range(NCORES)), **run_kwargs)
    outs = [res.results[c]["out"] for c in range(NCORES)]
    return _combine(outs, inv_lambda_diag), res


def kernel(input_data, weight, inv_lambda_diag):
    ans, _ = run(input_data, weight, inv_lambda_diag)
    return ans
